# revision 1
# baseline (speedup 1.0000x reference)
"""MCAM kernel (per-core program), v3.

Per core (one sample b):
  f_b = W_b @ x_b   (1x1 conv, fp32r matmuls, f32 PSUM) -> f16_b fp16 [c | pix]
  G   = PE-transpose of f16 (fp16, 1 cyc/row)            [(h, chalf) | k, w]
  S_c = F_c^T F_c   (fp16 gram, f32 PSUM)                [(i, chalf) | j, c-slot]
      c-slot is the INNERMOST (contiguous) free dim so per-j softmax ops
      run dense: DVE chunked max-reduce, then per-j ACT
      E = exp(S - M) with bias=-M and accum_out=Z (no separate sub/sum passes)
  had = E_o * E_s (fp16, in-place into E_o);  had2 = Square(had * rc) via ACT scale
  PE-transpose had2 back to [c | pix] (no DRAM round trip)
  att = (had2 * f16_o) * f16_s  (f32 out)

No DRAM spills at all; S + E stay in SBUF, branches processed sequentially.
"""
from contextlib import ExitStack

import numpy as np

import concourse.bass as bass
import concourse.bacc as bacc
import concourse.mybir as mybir
import concourse.tile as tile
from concourse.masks import make_identity

F32 = mybir.dt.float32
F32R = mybir.dt.float32r
F16 = mybir.dt.float16
AL = mybir.AluOpType
AF = mybir.ActivationFunctionType
AX = mybir.AxisListType

C, HH, WW = 512, 64, 64
PIX = HH * WW  # 4096
NM = 4
NK = 4
NSLAB = 8
PITCH = 64 * 256  # S free-pitch per partition: [j 64, c-slot 256]


def rap(t, dims, off=0):
    return bass.AP(tensor=t.tensor, offset=t.offset + off, ap=[list(d) for d in dims])


def build_core():
    nc = bacc.Bacc("TRN2", target_bir_lowering=False, debug=False)
    x_dram = {
        "o": nc.dram_tensor("x_opt", [C, PIX], F32R, kind="ExternalInput").ap(),
        "s": nc.dram_tensor("x_sar", [C, PIX], F32R, kind="ExternalInput").ap(),
    }
    w_dram = {
        "o": nc.dram_tensor("w_opt", [C, C], F32, kind="ExternalInput").ap(),
        "s": nc.dram_tensor("w_sar", [C, C], F32, kind="ExternalInput").ap(),
    }
    att = nc.dram_tensor("att", [C, PIX], F32, kind="ExternalOutput").ap()

    with tile.TileContext(nc) as tc, ExitStack() as ctx:
        persist = ctx.enter_context(tc.tile_pool(name="persist", bufs=1))
        smalls = ctx.enter_context(tc.tile_pool(name="smalls", bufs=1))
        cps = ctx.enter_context(tc.tile_pool(name="cps", bufs=2, space="PSUM"))
        tps = ctx.enter_context(tc.tile_pool(name="tps", bufs=2, space="PSUM"))
        gps = ctx.enter_context(tc.tile_pool(name="gps", bufs=2, space="PSUM"))

        ident = persist.tile([128, 128], F32, name="ident")
        make_identity(nc, ident)
        ident16 = persist.tile([128, 128], F16, name="ident16")
        make_identity(nc, ident16)
        f16 = {
            "o": persist.tile([128, NM, PIX], F16, name="f16_o"),
            "s": persist.tile([128, NM, PIX], F16, name="f16_s"),
        }
        had = persist.tile([128, 64, 256], F16, name="had")
        Zp = {
            "o": smalls.tile([128, 64], F32, name="Zp_o"),
            "s": smalls.tile([128, 64], F32, name="Zp_s"),
        }

        def load_wt(b, pool):
            """WT[ci_p, k, co] = W[co, k*128+ci_p]"""
            WT = pool.tile([128, NK, C], F32R, tag="WT")
            wsb = pool.tile([128, NM, C], F32, tag="wsb")
            nc.sync.dma_start(
                out=wsb, in_=w_dram[b].rearrange("(m p) ci -> p m ci", p=128)
            )
            for ko in range(NK):
                wps = cps.tile([128, C], F32, tag="cp")
                for mo in range(NM):
                    nc.tensor.transpose(
                        wps[:, mo * 128:(mo + 1) * 128],
                        in_=wsb[:, mo, ko * 128:(ko + 1) * 128],
                        identity=ident,
                    )
                nc.scalar.copy(out=WT[:, ko, :], in_=wps)
            return WT

        def conv(b, f_out, WT, pool, evac="scalar"):
            for slab in range(NSLAB):
                xt = pool.tile([128, NK, 512], F32R, tag="xt")
                for k in range(NK):
                    nc.sync.dma_start(
                        out=xt[:, k, :],
                        in_=x_dram[b][k * 128:(k + 1) * 128,
                                      slab * 512:(slab + 1) * 512],
                    )
                for m in range(NM):
                    cp = cps.tile([128, 512], F32, tag="cp")
                    for k in range(NK):
                        nc.tensor.matmul(
                            cp,
                            lhsT=WT[:, k, m * 128:(m + 1) * 128],
                            rhs=xt[:, k, :],
                            start=(k == 0),
                            stop=(k == NK - 1),
                        )
                    sl = slice(slab * 512, (slab + 1) * 512)
                    if evac == "scalar":
                        nc.scalar.copy(out=f_out[:, m, sl], in_=cp)
                    else:
                        nc.vector.tensor_copy(out=f_out[:, m, sl], in_=cp)

        def transpose_gram(b, f_raw, S, gpool):
            """S[(h2,i) | j, c-slot] with c = h2*256 + slot, slot = mpar*128+kl."""
            for mpar in range(2):
                G = gpool.tile([128, 128, WW], F16, tag="G")
                for wq in range(16):
                    for half, m in ((0, mpar), (1, mpar + 2)):
                        tp = tps.tile([64, 512], F16, tag="tp")
                        for wi in range(4):
                            w = wq * 4 + wi
                            src = rap(
                                f_raw[:, m, :], [[NM * PIX, 128], [WW, HH]], off=w
                            )
                            nc.tensor.transpose(
                                tp[:, wi * 128:(wi + 1) * 128],
                                in_=src,
                                identity=ident16,
                            )
                        # tp (64p=h, (wi 4 @128, cl 128 @1)) -> G[half, cl, wq*4+wi]
                        dst = rap(
                            G[half * 64:(half + 1) * 64, :, :],
                            [[128 * WW, 64], [WW, 128], [1, 4]],
                            off=wq * 4,
                        )
                        srcap = rap(tp, [[512, 64], [1, 128], [128, 4]])
                        nc.scalar.copy(out=dst, in_=srcap)
                for kg in range(16):
                    gp = gps.tile([128, 512], F32, tag="gp")
                    for sl in range(8):
                        kl = kg * 8 + sl
                        a0 = G[0:64, kl, :]
                        nc.tensor.matmul(
                            gp[0:64, sl * 64:(sl + 1) * 64],
                            lhsT=a0, rhs=a0, start=True, stop=True,
                        )
                        a1 = G[64:128, kl, :]
                        nc.tensor.matmul(
                            gp[64:128, sl * 64:(sl + 1) * 64],
                            lhsT=a1, rhs=a1, start=True, stop=True,
                        )
                    # gp[p, sl*64+j] -> S[p, j, k0+sl]  (c contiguous innermost)
                    k0 = mpar * 128 + kg * 8
                    nc.vector.tensor_copy(
                        out=rap(S, [[PITCH, 128], [256, 64], [1, 8]], off=k0),
                        in_=rap(gp, [[512, 128], [1, 64], [64, 8]]),
                    )

        def softmax_fused(b, S, epool, mxp):
            """E = exp(S - max_c S) per (i, j); Z sums via ACT accum_out.

            Branch o writes E into `had`; branch s multiplies into `had`.
            """
            for jc in range(4):
                j0 = jc * 16
                Mp = mxp.tile([128, 16], F32, tag="Mp")
                nc.vector.tensor_reduce(
                    out=Mp,
                    in_=rap(S, [[PITCH, 128], [256, 16], [1, 256]], off=j0 * 256),
                    axis=AX.X,
                    op=AL.max,
                )
                tmp = mxp.tile([64, 16], F32, tag="tmp")
                nc.scalar.copy(out=tmp, in_=Mp[64:128])
                nMx = mxp.tile([128, 16], F32, tag="nMx")
                nc.vector.tensor_tensor(
                    out=nMx[0:64], in0=Mp[0:64], in1=tmp, op=AL.max
                )
                nc.vector.tensor_scalar_mul(
                    out=nMx[0:64], in0=nMx[0:64], scalar1=-1.0
                )
                nc.scalar.copy(out=nMx[64:128], in_=nMx[0:64])
                for j in range(16):
                    jj = j0 + j
                    src = S[:, jj, :]
                    if b == "o":
                        nc.scalar.activation(
                            out=had[:, jj, :],
                            in_=src,
                            func=AF.Exp,
                            bias=nMx[:, j:j + 1],
                            accum_out=Zp[b][:, jj:jj + 1],
                        )
                    else:
                        eb = epool.tile([128, 256], F16, tag="eb")
                        nc.scalar.activation(
                            out=eb,
                            in_=src,
                            func=AF.Exp,
                            bias=nMx[:, j:j + 1],
                            accum_out=Zp[b][:, jj:jj + 1],
                        )
                        nc.vector.tensor_tensor(
                            out=had[:, jj, :], in0=had[:, jj, :], in1=eb,
                            op=AL.mult,
                        )

        # ================= schedule =================
        for b in "os":
            with tc.tile_pool(name=f"xw_{b}", bufs=2) as xw:
                WT = load_wt(b, xw)
                conv(b, f16[b], WT, xw)
            with (
                tc.tile_pool(name=f"sg_{b}", bufs=1) as sg,
                tc.tile_pool(name=f"gpool_{b}", bufs=1) as gpool,
                tc.tile_pool(name=f"ep_{b}", bufs=2) as epool,
                tc.tile_pool(name=f"mx_{b}", bufs=2) as mxp,
            ):
                S = sg.tile([128, 64, 256], F32, tag="S")
                transpose_gram(b, f16[b], S, gpool)
                softmax_fused(b, S, epool, mxp)

        # 1/(Z_o * Z_s), folded into Square via ACT scale
        Zt = {}
        for b in "os":
            t2 = smalls.tile([64, 64], F32, name=f"zt2_{b}")
            nc.scalar.copy(out=t2, in_=Zp[b][64:128])
            Zt[b] = smalls.tile([64, 64], F32, name=f"zt_{b}")
            nc.vector.tensor_tensor(out=Zt[b], in0=Zp[b][0:64], in1=t2, op=AL.add)
        zz = smalls.tile([64, 64], F32, name="zz")
        nc.vector.tensor_tensor(out=zz, in0=Zt["o"], in1=Zt["s"], op=AL.mult)
        rc2 = smalls.tile([128, 64], F32, name="rc2")
        nc.vector.reciprocal(out=rc2[0:64], in_=zz)
        nc.scalar.copy(out=rc2[64:128], in_=rc2[0:64])

        # had2 = (had * rc)^2, in place, per-j so rc is a per-partition scalar
        for jj in range(64):
            nc.scalar.activation(
                out=had[:, jj, :], in_=had[:, jj, :], func=AF.Square,
                scale=rc2[:, jj:jj + 1],
            )

        # PE-transpose had2 [(h2,i) | j, slot] -> hc [c | i, j] and combine:
        # att = (had2 * f_o) * f_s.  c = h2*256 + sb*128 + p  => m = h2*2 + sb.
        with (
            tc.tile_pool(name="hcp", bufs=2) as hcp,
            tc.tile_pool(name="apool", bufs=2) as apool,
            tc.tile_pool(name="tops", bufs=2, space="PSUM") as tops,
        ):
            for m in range(NM):
                h2, sb = m // 2, m % 2
                hc = hcp.tile([128, PIX], F16, tag="hc")
                for jg in range(8):
                    tpo = tops.tile([128, 8, 64], F16, tag="tpo")
                    for j8 in range(8):
                        j = jg * 8 + j8
                        nc.tensor.transpose(
                            tpo[:, j8, :],
                            in_=had[h2 * 64:(h2 + 1) * 64, j,
                                    sb * 128:(sb + 1) * 128],
                            identity=ident16[h2 * 64:(h2 + 1) * 64,
                                             h2 * 64:(h2 + 1) * 64],
                        )
                    # tpo[p, j8, i] -> hc[p, i*64 + jg*8 + j8]
                    dst = rap(hc, [[PIX, 128], [1, 8], [64, 64]], off=jg * 8)
                    srcap = rap(tpo, [[512, 128], [64, 8], [1, 64]])
                    if jg % 2 == 0:
                        nc.scalar.copy(out=dst, in_=srcap)
                    else:
                        nc.vector.tensor_copy(out=dst, in_=srcap)
                for ih in range(2):
                    psl = slice(ih * 2048, (ih + 1) * 2048)
                    vv = apool.tile([128, PIX // 2], F16, tag="vv")
                    nc.vector.tensor_tensor(
                        out=vv, in0=hc[:, psl], in1=f16["o"][:, m, psl], op=AL.mult
                    )
                    at = apool.tile([128, PIX // 2], F32, tag="at")
                    nc.vector.tensor_tensor(
                        out=at, in0=vv, in1=f16["s"][:, m, psl], op=AL.mult
                    )
                    nc.sync.dma_start(
                        out=att[m * 128:(m + 1) * 128, psl], in_=at
                    )

    nc.compile()
    return nc


_NC_CACHE = {}


def _get_nc():
    if "nc" not in _NC_CACHE:
        _NC_CACHE["nc"] = build_core()
    return _NC_CACHE["nc"]


def kernel(opt, sar, W_opt, W_sar):
    """Full inputs (8,512,64,64)x2 + (512,512)x2 -> full output (8,512,64,64).

    Data-parallel over batch: one sample per NeuronCore.
    """
    from concourse.bass_utils import run_bass_kernel_spmd

    B = opt.shape[0]
    nc = _get_nc()
    in_maps = [
        {
            "x_opt": np.ascontiguousarray(opt[b].reshape(C, PIX), dtype=np.float32),
            "x_sar": np.ascontiguousarray(sar[b].reshape(C, PIX), dtype=np.float32),
            "w_opt": np.ascontiguousarray(W_opt, dtype=np.float32),
            "w_sar": np.ascontiguousarray(W_sar, dtype=np.float32),
        }
        for b in range(B)
    ]
    res = run_bass_kernel_spmd(nc, in_maps, core_ids=list(range(B)))
    out = np.stack([res.results[b]["att"].reshape(C, HH, WW) for b in range(B)])
    return out.astype(np.float32)



# revision 17
# speedup vs baseline: 1.0128x; 1.0128x over previous
"""MCAM kernel (per-core program), v4.

Per core (one sample b):
  f_b = W_b @ x_b   (1x1 conv, fp32r matmuls, f32 PSUM) -> f16_b fp16 [c | pix]
  G tiles (per mpar): [p=(mhalf,h) | w 64, c_l 128] built via PAIRED PE
      transposes (tile_position col groups 0/64) -> full-128-partition
      CONTIGUOUS fp16 PSUM->SBUF evacs.
  Gram: 4 concurrent MMs per channel-step (quadrant tile_positions),
      S[p=(half,i) | j 64, slot 256]; channel c = m*128 + (slot%128) with
      m = (slot//128)*2 + half.  Running max over c computed from the gram
      PSUM tiles (DVE tensor_reduce) so softmax starts right after gram.
  Softmax: E = exp(S - M) on ACT (bias fused, accum_out -> Z); branch o
      writes `had`; branch s exp -> eb chunks, DVE TT multiplies into had.
  (had*rc)^2: DVE TS (had*rc -> eb, reusing the chunk tile) + DVE TT square
      back into had.
  PE-transpose had2 back to [c | pix] (row-half concurrent), combine
      att = (had2 * f16_o) * f16_s in fp16, DMA out fp16 (host casts f32).

f16_o/f16_s spill to DRAM after their G-transposes and reload before the
combine so S_o + S_s + G + had fit in SBUF; this lets conv_s/gtp_s/gram_s
(PE) overlap softmax_o (ACT/DVE).  Long-lived tiles (S_o, S_s, f16r, hc)
live on the RIGHT SBUF stack; phase pools on the LEFT.
"""
import os
from contextlib import ExitStack

import numpy as np

import concourse.bass as bass
import concourse.bacc as bacc
import concourse.mybir as mybir
import concourse.tile as tile
from concourse.masks import make_identity

F32 = mybir.dt.float32
F32R = mybir.dt.float32r
F16 = mybir.dt.float16
AL = mybir.AluOpType
AF = mybir.ActivationFunctionType
AX = mybir.AxisListType

C, HH, WW = 512, 64, 64
PIX = HH * WW  # 4096
NM = 4
NK = 4
NSLAB = 8
PITCH = 64 * 256  # S free-pitch per partition: [j 64, slot 256]

# bisect flags (defaults = shipping config)
TPPAIR = os.environ.get("K_TPPAIR", "1") == "1"
GRAM4 = os.environ.get("K_GRAM4", "1") == "1"
OUT16 = os.environ.get("K_OUT16", "1") == "1"


def rap(t, dims, off=0):
    return bass.AP(tensor=t.tensor, offset=t.offset + off, ap=[list(d) for d in dims])


def build_core():
    nc = bacc.Bacc("TRN2", target_bir_lowering=False, debug=False)
    x_dram = {
        "o": nc.dram_tensor("x_opt", [C, PIX], F32R, kind="ExternalInput").ap(),
        "s": nc.dram_tensor("x_sar", [C, PIX], F32R, kind="ExternalInput").ap(),
    }
    w_dram = {
        "o": nc.dram_tensor("w_opt", [C, C], F32, kind="ExternalInput").ap(),
        "s": nc.dram_tensor("w_sar", [C, C], F32, kind="ExternalInput").ap(),
    }
    att = nc.dram_tensor(
        "att", [C, PIX], F16 if OUT16 else F32, kind="ExternalOutput"
    ).ap()

    with tile.TileContext(nc) as tc, ExitStack() as ctx:
        persist = ctx.enter_context(tc.tile_pool(name="persist", bufs=1))
        smalls = ctx.enter_context(tc.tile_pool(name="smalls", bufs=1))
        cps = ctx.enter_context(tc.tile_pool(name="cps", bufs=2, space="PSUM"))

        ident = persist.tile([128, 128], F32, name="ident")
        make_identity(nc, ident)
        ident16 = persist.tile([128, 128], F16, name="ident16")
        make_identity(nc, ident16)

        # had: E_o, then E_o*E_s, finally (had*rc)^2, per chunk
        had = persist.tile([128, 64, 256], F16, name="had")
        Zp = {
            "o": smalls.tile([128, 64], F32, name="Zp_o"),
            "s": smalls.tile([128, 64], F32, name="Zp_s"),
        }
        Mrun = {
            "o": smalls.tile([128, 64], F32, name="Mrun_o"),
            "s": smalls.tile([128, 64], F32, name="Mrun_s"),
        }
        negM = {
            "o": smalls.tile([128, 64], F32, name="negM_o"),
            "s": smalls.tile([128, 64], F32, name="negM_s"),
        }
        Mpart = {
            "o": smalls.tile([128, 64], F32, name="Mpart_o"),
            "s": smalls.tile([128, 64], F32, name="Mpart_s"),
        }
        Zt_o = smalls.tile([64, 64], F32, name="Zt_o")

        def load_wt(b, pool):
            """WT[ci_p, k, co] = W[co, k*128+ci_p]"""
            WT = pool.tile([128, NK, C], F32R, tag="WT")
            wsb = pool.tile([128, NM, C], F32, tag="wsb")
            nc.sync.dma_start(
                out=wsb, in_=w_dram[b].rearrange("(m p) ci -> p m ci", p=128)
            )
            for ko in range(NK):
                wps = cps.tile([128, C], F32, tag="cp")
                for mo in range(NM):
                    nc.tensor.transpose(
                        wps[:, mo * 128:(mo + 1) * 128],
                        in_=wsb[:, mo, ko * 128:(ko + 1) * 128],
                        identity=ident,
                    )
                nc.scalar.copy(out=WT[:, ko, :], in_=wps)
            return WT

        def conv(b, f_out, WT, pool, evac_acts):
            """evac_acts: number of slabs (of NSLAB) evacuated on ACT."""
            for slab in range(NSLAB):
                xt = pool.tile([128, NK, 512], F32R, tag="xt")
                for k in range(NK):
                    nc.sync.dma_start(
                        out=xt[:, k, :],
                        in_=x_dram[b][k * 128:(k + 1) * 128,
                                      slab * 512:(slab + 1) * 512],
                    )
                for m in range(NM):
                    cp = cps.tile([128, 512], F32, tag="cp")
                    for k in range(NK):
                        nc.tensor.matmul(
                            cp,
                            lhsT=WT[:, k, m * 128:(m + 1) * 128],
                            rhs=xt[:, k, :],
                            start=(k == 0),
                            stop=(k == NK - 1),
                        )
                    sl = slice(slab * 512, (slab + 1) * 512)
                    if slab < evac_acts:
                        nc.scalar.copy(out=f_out[:, m, sl], in_=cp)
                    else:
                        nc.vector.tensor_copy(out=f_out[:, m, sl], in_=cp)

        def gtranspose(b, f16_t, G2, tps, dve_evacs):
            """G2[mpar][p=(mhalf,h) | w 64, c_l 128]; mhalf 0 -> m=mpar,
            mhalf 1 -> m=mpar+2.  Paired transposes (col groups 0/64) fill
            all 128 partitions; evacs contiguous fp16 [128, 1024]."""
            step = 0
            for mpar in range(2):
                for wq in range(8):
                    if TPPAIR:
                        tp = tps.tile([128, 8, 128], F16, tag="tp")
                        for wi in range(8):
                            w = wq * 8 + wi
                            for half, m in ((0, mpar), (1, mpar + 2)):
                                src = rap(
                                    f16_t[:, m, :], [[NM * PIX, 128], [WW, HH]],
                                    off=w,
                                )
                                nc.tensor.transpose(
                                    tp[half * 64:(half + 1) * 64, wi, :],
                                    in_=src,
                                    identity=ident16,
                                    tile_position=(0, half * 64),
                                )
                        dst = G2[mpar][:, wq * 8:(wq + 1) * 8, :]
                        if (step % 4) < dve_evacs:
                            nc.vector.tensor_copy(out=dst, in_=tp)
                        else:
                            nc.scalar.copy(out=dst, in_=tp)
                    else:
                        tph = [
                            tps.tile([64, 8, 128], F16, tag=f"tp{h}", name=f"tp{h}")
                            for h in range(2)
                        ]
                        for wi in range(8):
                            w = wq * 8 + wi
                            for half, m in ((0, mpar), (1, mpar + 2)):
                                src = rap(
                                    f16_t[:, m, :], [[NM * PIX, 128], [WW, HH]],
                                    off=w,
                                )
                                nc.tensor.transpose(
                                    tph[half][:, wi, :],
                                    in_=src,
                                    identity=ident16,
                                )
                        for half in range(2):
                            dst = G2[mpar][half * 64:(half + 1) * 64,
                                           wq * 8:(wq + 1) * 8, :]
                            if (step % 4) < dve_evacs:
                                nc.vector.tensor_copy(out=dst, in_=tph[half])
                            else:
                                nc.scalar.copy(out=dst, in_=tph[half])
                    step += 1

        def gram(b, G2, S, gps, act_evac_first):
            """4 concurrent MMs (quadrants) per channel step; running max
            over c taken from the PSUM tiles so M is ready at gram end."""
            Mr, Mp = Mrun[b], Mpart[b]
            for grp in range(16):
                gp = gps.tile([128, 16, 64], F32, tag="gp")
                for t in range(8):
                    kl = grp * 8 + t
                    for gidx in range(2):
                        for mh in range(2):
                            lhs = G2[gidx][mh * 64:(mh + 1) * 64, :, kl]
                            if GRAM4:
                                # S-half = gidx, slot-block = mh; all 4
                                # quadrants used -> 4 concurrent MMs
                                nc.tensor.matmul(
                                    gp[gidx * 64:(gidx + 1) * 64, 2 * t + mh, :],
                                    lhsT=lhs,
                                    rhs=lhs,
                                    start=True,
                                    stop=True,
                                    tile_position=(mh * 64, gidx * 64),
                                )
                            else:
                                # S-half = mh, slot-block = gidx; diagonal
                                # quadrants only (auto tile_position)
                                nc.tensor.matmul(
                                    gp[mh * 64:(mh + 1) * 64, 2 * t + gidx, :],
                                    lhsT=lhs,
                                    rhs=lhs,
                                    start=True,
                                    stop=True,
                                )
                tr_in = rap(gp, [[1024, 128], [1, 64], [64, 16]])
                if grp == 0:
                    nc.vector.tensor_reduce(out=Mr, in_=tr_in, axis=AX.X, op=AL.max)
                else:
                    nc.vector.tensor_reduce(out=Mp, in_=tr_in, axis=AX.X, op=AL.max)
                    nc.vector.tensor_tensor(out=Mr, in0=Mr, in1=Mp, op=AL.max)
                for mh in range(2):
                    src = rap(gp, [[1024, 128], [128, 8], [1, 64]], off=mh * 64)
                    dst = rap(
                        S, [[PITCH, 128], [1, 8], [256, 64]],
                        off=mh * 128 + grp * 8,
                    )
                    if (grp % 2 == 0) == act_evac_first:
                        nc.scalar.copy(out=dst, in_=src)
                    else:
                        nc.vector.tensor_copy(out=dst, in_=src)

        def build_negM(b):
            tmp = smalls.tile([64, 64], F32, name=f"nmt_{b}")
            nc.scalar.copy(out=tmp, in_=Mrun[b][64:128])
            nc.vector.tensor_tensor(
                out=negM[b][0:64], in0=Mrun[b][0:64], in1=tmp, op=AL.max
            )
            nc.vector.tensor_scalar_mul(
                out=negM[b][0:64], in0=negM[b][0:64], scalar1=-1.0
            )
            nc.scalar.copy(out=negM[b][64:128], in_=negM[b][0:64])

        # ================= schedule =================
        f16p = ctx.enter_context(tc.tile_pool(name="f16p", bufs=1))
        f16 = {
            "o": f16p.tile([128, NM, PIX], F16, name="f16_o"),
            "s": f16p.tile([128, NM, PIX], F16, name="f16_s"),
        }

        # --- branch o: conv, G-transpose, gram ---
        So_cm = tc.tile_pool(name="S_o", bufs=1, side="right")
        S_o = So_cm.__enter__().tile([128, 64, 256], F32, name="S_o")

        with tc.tile_pool(name="xw_o", bufs=2) as xw:
            WT = load_wt("o", xw)
            conv("o", f16["o"], WT, xw, evac_acts=4)
        with (
            tc.tile_pool(name="G_o", bufs=1) as gpool_o,
            tc.tile_pool(name="tps_o", bufs=2 if TPPAIR else 1, space="PSUM") as tps_o,
        ):
            G2o = [
                gpool_o.tile([128, 64, 128], F16, name=f"G_o{i}")
                for i in range(2)
            ]
            gtranspose("o", f16["o"], G2o, tps_o, dve_evacs=2)
            with tc.tile_pool(name="gps_o", bufs=2, space="PSUM") as gps_o:
                gram("o", G2o, S_o, gps_o, act_evac_first=True)

        # --- branch s conv/gtp/gram (PE) overlap softmax_o (ACT/DVE) ---
        with tc.tile_pool(name="xw_s", bufs=2) as xw:
            WT = load_wt("s", xw)
            conv("s", f16["s"], WT, xw, evac_acts=0)  # DVE evacs; ACT on exp_o

        # softmax_o on ACT (+ a little DVE), overlapping the PE work
        build_negM("o")
        for j in range(64):
            nc.scalar.activation(
                out=had[:, j, :],
                in_=S_o[:, j, :],
                func=AF.Exp,
                bias=negM["o"][:, j:j + 1],
                accum_out=Zp["o"][:, j:j + 1],
            )
        t2 = smalls.tile([64, 64], F32, name="zt2_o")
        nc.scalar.copy(out=t2, in_=Zp["o"][64:128])
        nc.vector.tensor_tensor(out=Zt_o, in0=Zp["o"][0:64], in1=t2, op=AL.add)

        with (
            tc.tile_pool(name="G_s", bufs=1) as gpool_s,
            tc.tile_pool(name="tps_s", bufs=2 if TPPAIR else 1, space="PSUM") as tps_s,
        ):
            G2s = [
                gpool_s.tile([128, 64, 128], F16, name=f"G_s{i}")
                for i in range(2)
            ]
            gtranspose("s", f16["s"], G2s, tps_s, dve_evacs=4)
            # S_o frees (exp_o is its last reader); S_s takes its place
            So_cm.__exit__(None, None, None)
            Ss_cm = tc.tile_pool(name="S_s", bufs=1, side="right")
            S_s = Ss_cm.__enter__().tile([128, 64, 256], F32, name="S_s")
            with tc.tile_pool(name="gps_s", bufs=2, space="PSUM") as gps_s:
                gram("s", G2s, S_s, gps_s, act_evac_first=False)

        hc_cm = tc.tile_pool(name="hc", bufs=1, side="right")
        hc_p = hc_cm.__enter__()
        hc = [hc_p.tile([128, PIX], F16, name=f"hc{m}") for m in range(NM)]

        # --- softmax_s + hadamard + (had*rc)^2 + transpose-back, chunked ---
        build_negM("s")
        with (
            tc.tile_pool(name="eb", bufs=2) as ebp,
            tc.tile_pool(name="tops", bufs=2, space="PSUM") as tops,
        ):
            for chk in range(8):  # chunks of 8 j
                j0 = chk * 8
                cols = slice(j0, j0 + 8)
                eb = ebp.tile([128, 8, 256], F16, tag="eb")
                for jj in range(8):
                    j = j0 + jj
                    nc.scalar.activation(
                        out=eb[:, jj, :],
                        in_=S_s[:, j, :],
                        func=AF.Exp,
                        bias=negM["s"][:, j:j + 1],
                        accum_out=Zp["s"][:, j:j + 1],
                    )
                hs = had[:, j0:j0 + 8, :]
                nc.vector.tensor_tensor(out=hs, in0=hs, in1=eb, op=AL.mult)
                # rc = 1/(Zo*Zs) for this chunk, both partition halves
                t2s = smalls.tile([64, 8], F32, name=f"t2s_{chk}")
                nc.scalar.copy(out=t2s, in_=Zp["s"][64:128, cols])
                zts = smalls.tile([64, 8], F32, name=f"zts_{chk}")
                nc.vector.tensor_tensor(
                    out=zts, in0=Zp["s"][0:64, cols], in1=t2s, op=AL.add
                )
                nc.vector.tensor_tensor(
                    out=zts, in0=zts, in1=Zt_o[:, cols], op=AL.mult
                )
                rc = smalls.tile([128, 8], F32, name=f"rc_{chk}")
                nc.vector.reciprocal(out=rc[0:64], in_=zts)
                nc.scalar.copy(out=rc[64:128], in_=rc[0:64])
                # had*rc -> eb (chunk tile is free now), square back into had
                for jj in range(8):
                    j = j0 + jj
                    nc.vector.tensor_scalar(
                        out=eb[:, jj, :],
                        in0=had[:, j, :],
                        scalar1=rc[:, jj:jj + 1],
                        scalar2=None,
                        op0=AL.mult,
                    )
                nc.vector.tensor_tensor(out=hs, in0=eb, in1=eb, op=AL.mult)
                # transpose-back this chunk: m = sb*2 + half
                for sb in range(2):
                    tpo = []
                    for h in range(2):
                        t_ = tops.tile([128, 8, 64], F16, tag=f"tpo{h}")
                        tpo.append(t_)
                    for jj in range(8):
                        j = j0 + jj
                        for half in range(2):
                            nc.tensor.transpose(
                                tpo[half][:, jj, :],
                                in_=had[half * 64:(half + 1) * 64, j,
                                        sb * 128:(sb + 1) * 128],
                                identity=ident16[half * 64:(half + 1) * 64,
                                                 half * 64:(half + 1) * 64],
                            )
                    for half in range(2):
                        m = sb * 2 + half if GRAM4 else half * 2 + sb
                        dst = rap(hc[m], [[PIX, 128], [1, 8], [64, 64]], off=j0)
                        srcap = rap(tpo[half], [[512, 128], [64, 8], [1, 64]])
                        if half == 0:
                            nc.scalar.copy(out=dst, in_=srcap)
                        else:
                            nc.vector.tensor_copy(out=dst, in_=srcap)

        # --- combine + DMA out (fp16; host casts to f32) ---
        cw = 2048 if OUT16 else 1024
        with tc.tile_pool(name="apool", bufs=2 if OUT16 else 1) as apool:
            for m in range(NM):
                for ih in range(PIX // cw):
                    psl = slice(ih * cw, (ih + 1) * cw)
                    vv = apool.tile([128, cw], F16, tag="vv")
                    nc.vector.tensor_tensor(
                        out=vv, in0=hc[m][:, psl], in1=f16["o"][:, m, psl],
                        op=AL.mult,
                    )
                    if OUT16:
                        nc.vector.tensor_tensor(
                            out=vv, in0=vv, in1=f16["s"][:, m, psl], op=AL.mult
                        )
                        src = vv
                    else:
                        at = apool.tile([128, cw], F32, tag="at")
                        nc.vector.tensor_tensor(
                            out=at, in0=vv, in1=f16["s"][:, m, psl],
                            op=AL.mult
                        )
                        src = at
                    nc.sync.dma_start(
                        out=att[m * 128:(m + 1) * 128, psl], in_=src
                    )

        hc_cm.__exit__(None, None, None)
        Ss_cm.__exit__(None, None, None)

    nc.compile()
    return nc


_NC_CACHE = {}


def _get_nc():
    if "nc" not in _NC_CACHE:
        _NC_CACHE["nc"] = build_core()
    return _NC_CACHE["nc"]


def kernel(opt, sar, W_opt, W_sar):
    """Full inputs (8,512,64,64)x2 + (512,512)x2 -> full output (8,512,64,64).

    Data-parallel over batch: one sample per NeuronCore.
    """
    from concourse.bass_utils import run_bass_kernel_spmd

    B = opt.shape[0]
    nc = _get_nc()
    in_maps = [
        {
            "x_opt": np.ascontiguousarray(opt[b].reshape(C, PIX), dtype=np.float32),
            "x_sar": np.ascontiguousarray(sar[b].reshape(C, PIX), dtype=np.float32),
            "w_opt": np.ascontiguousarray(W_opt, dtype=np.float32),
            "w_sar": np.ascontiguousarray(W_sar, dtype=np.float32),
        }
        for b in range(B)
    ]
    res = run_bass_kernel_spmd(nc, in_maps, core_ids=list(range(B)))
    out = np.stack([res.results[b]["att"].reshape(C, HH, WW) for b in range(B)])
    return out.astype(np.float32)
